# revision 1
# baseline (speedup 1.0000x reference)
"""Banded multi-head attention on 8 Trainium2 NeuronCores.

Problem: B=2, L=2048, D=1024, H=16 heads, d_k=64. The band mask is a 0/1
FLOAT tensor *added* to the scores (not -inf masked), so the softmax is
dense; exp(s + band) = exp(s) * e^band is handled by multiplying constant
e-or-1 parallelogram tiles over the band region.

Sharding: (batch x 4-head-groups) across the 8 cores. Host pre-transposes
activations/weights so every device matmul has its contraction dim on
partitions; the per-core partial output projections are summed on the host
(gather/unshard) together with the output bias.

All device matmuls run in float32r (TF32-like fast PE path, ~1.5e-4 rel).
"""

import sys

sys.path.insert(0, "/opt/trn_rl_repo")

import numpy as np
import ml_dtypes
from contextlib import ExitStack

import concourse.bass as bass
import concourse.tile as tile
from concourse import bacc, mybir
from concourse.bass_utils import run_bass_kernel_spmd

dt = mybir.dt
AF = mybir.ActivationFunctionType
bf16 = dt.bfloat16

B, L, D, H, DK = 2, 2048, 1024, 16, 64
HPC = 4            # heads per core
HD = HPC * DK      # 256: head dims per core
NQC, QCW = 4, 512  # q chunks
NKB, KBW = 16, 128 # k blocks
NDC, DCW = 8, 128  # D chunks
SCALE = 1.0 / 8.0  # 1/sqrt(d_k)

_CACHE = {}


def _band_slots(half):
    """delta -> (slot, c0, c1) for 128x512 tiles at k-offset kb*128, q-offset
    qc*512, delta = kb*128 - qc*512. Band cols: f in [delta-half, delta+127+half]."""
    slots = {}
    d = -((half + 127) // 128) * 128
    while d <= half + 511:
        c0, c1 = max(0, d - half), min(512, d + 128 + half)
        if c0 < c1:
            slots[d] = (len(slots), c0, c1)
        d += 128
    return slots


def _build(masksize, stop_after=None):
    half = int(masksize) // 2
    slots = _band_slots(half)
    ns = max(len(slots), 1)

    nc = bacc.Bacc("TRN2", target_bir_lowering=False, debug=False)

    f32, f32r = dt.float32, dt.float32r
    xq = nc.dram_tensor("xq", [D, L], f32, kind="ExternalInput").ap()
    xk = nc.dram_tensor("xk", [D, L], f32, kind="ExternalInput").ap()
    xv = nc.dram_tensor("xv", [D, L], f32, kind="ExternalInput").ap()
    # weights pre-packed on host into SBUF layouts (see _prep_inmaps)
    wq = nc.dram_tensor("wq", [128, NDC * HD], f32, kind="ExternalInput").ap()
    wk = nc.dram_tensor("wk", [128, NDC * HD], f32, kind="ExternalInput").ap()
    wv = nc.dram_tensor("wv", [128, NDC * HD], f32, kind="ExternalInput").ap()
    wo = nc.dram_tensor("wo", [64, HPC * D], f32, kind="ExternalInput").ap()
    bq = nc.dram_tensor("bq", [128, 2], f32, kind="ExternalInput").ap()
    bk = nc.dram_tensor("bk", [128, 2], f32, kind="ExternalInput").ap()
    bv = nc.dram_tensor("bv", [128, HD + 2], f32, kind="ExternalInput").ap()
    em = nc.dram_tensor("em", [128, ns * 512], f32, kind="ExternalInput").ap()
    on1 = nc.dram_tensor("on1", [1, 64], f32, kind="ExternalInput").ap()
    yt = nc.dram_tensor("yt", [D, L], f32, kind="ExternalOutput").ap()

    with tile.TileContext(nc) as tc, ExitStack() as ctx:
        ctx.enter_context(
            nc.allow_low_precision(reason="fp32r matmul operands are intentional")
        )
        # ---- persistent SBUF ----
        wts = ctx.enter_context(tc.tile_pool(name="wts", bufs=1))
        big = ctx.enter_context(tc.tile_pool(name="big", bufs=1))

        wq_sb = wts.tile([128, NDC * HD], f32r, tag="wq", name="wq")
        wk_sb = wts.tile([128, NDC * HD], f32r, tag="wk", name="wk")
        wv_sb = wts.tile([128, NDC * HD], f32r, tag="wv", name="wv")
        wo_sb = wts.tile([64, HPC * D], f32r, tag="wo", name="wo")
        for w_sb, w in ((wq_sb, wq), (wk_sb, wk), (wv_sb, wv), (wo_sb, wo)):
            nc.sync.dma_start(w_sb[:], w[:].bitcast(f32r))
        bq_sb = wts.tile([128, 2], f32, tag="bq", name="bq")
        bk_sb = wts.tile([128, 2], f32, tag="bk", name="bk")
        bv_sb = wts.tile([128, HD + 2], f32, tag="bv", name="bv")
        em_sb = wts.tile([128, ns * 512], f32, tag="em", name="em")
        for t_sb, t_in in ((bq_sb, bq), (bk_sb, bk), (bv_sb, bv), (em_sb, em)):
            nc.sync.dma_start(t_sb[:], t_in[:])
        on1_sb = wts.tile([1, 64], f32r, tag="on1", name="on1")
        nc.sync.dma_start(on1_sb[:], on1[:].bitcast(f32r))

        # projection outputs (resident): 2 tiles each of [128, L]
        qt_sb = [big.tile([128, L], f32r, tag=f"qt{t}", name=f"qt{t}") for t in range(2)]
        kt_sb = [big.tile([128, L], f32r, tag=f"kt{t}", name=f"kt{t}") for t in range(2)]
        ot_sb = [big.tile([64, L], f32r, tag=f"ot{h}", name=f"ot{h}") for h in range(HPC)]
        # v (natural layout) + ones col per head: [128, HPC*66] per k-block
        vaug_sb = [big.tile([128, HPC * 66], f32r, tag=f"vaug{lb}", name=f"vaug{lb}") for lb in range(NKB)]

        # ---- phase B: q/k projections (T-layout) ----
        with tc.tile_pool(name="xs", bufs=3) as xs, \
             tc.tile_pool(name="pqk", bufs=2, space="PSUM") as pqk:
            for qc in range(NQC):
                pq = [pqk.tile([128, QCW], f32, tag=f"pq{t}", name=f"pq{t}") for t in range(2)]
                pk = [pqk.tile([128, QCW], f32, tag=f"pk{t}", name=f"pk{t}") for t in range(2)]
                for c in range(NDC):
                    xq_t = xs.tile([128, QCW], f32r, tag="xq", name="xq")
                    nc.sync.dma_start(
                        xq_t[:], xq[c * DCW:(c + 1) * DCW, qc * QCW:(qc + 1) * QCW].bitcast(f32r)
                    )
                    xk_t = xs.tile([128, QCW], f32r, tag="xk", name="xk")
                    nc.sync.dma_start(
                        xk_t[:], xk[c * DCW:(c + 1) * DCW, qc * QCW:(qc + 1) * QCW].bitcast(f32r)
                    )
                    for t in range(2):
                        nc.tensor.matmul(
                            pq[t][:], wq_sb[:, c * HD + t * 128: c * HD + (t + 1) * 128],
                            xq_t[:], start=(c == 0), stop=(c == NDC - 1),
                        )
                        nc.tensor.matmul(
                            pk[t][:], wk_sb[:, c * HD + t * 128: c * HD + (t + 1) * 128],
                            xk_t[:], start=(c == 0), stop=(c == NDC - 1),
                        )
                for t in range(2):
                    nc.scalar.activation(
                        qt_sb[t][:, qc * QCW:(qc + 1) * QCW], pq[t][:],
                        AF.Identity, bias=bq_sb[:, t:t + 1],
                    )
                    nc.scalar.activation(
                        kt_sb[t][:, qc * QCW:(qc + 1) * QCW], pk[t][:],
                        AF.Identity, bias=bk_sb[:, t:t + 1],
                    )

        # ---- phase C: v in natural [L, HD] layout, +bias, +ones col ----
        with tc.tile_pool(name="xvp", bufs=3) as xvs, \
             tc.tile_pool(name="pvp", bufs=2, space="PSUM") as pvp:
            for lg in range(4):  # groups of 4 k-blocks
                pv = [pvp.tile([128, HD], f32, tag=f"pv{j}", name=f"pv{j}") for j in range(4)]
                for c in range(NDC):
                    xv_t = xvs.tile([128, QCW], f32r, tag="xv", name="xv")
                    nc.sync.dma_start(
                        xv_t[:], xv[c * DCW:(c + 1) * DCW, lg * QCW:(lg + 1) * QCW].bitcast(f32r)
                    )
                    for j in range(4):
                        nc.tensor.matmul(
                            pv[j][:], xv_t[:, j * 128:(j + 1) * 128],
                            wv_sb[:, c * HD:(c + 1) * HD],
                            start=(c == 0), stop=(c == NDC - 1),
                        )
                for j in range(4):
                    lb = lg * 4 + j
                    for h in range(HPC):
                        nc.vector.tensor_add(
                            vaug_sb[lb][:, h * 66: h * 66 + 64],
                            pv[j][:, h * DK:(h + 1) * DK],
                            bv_sb[:, h * DK:(h + 1) * DK],
                        )
                        nc.vector.tensor_copy(
                            vaug_sb[lb][:, h * 66 + 64: h * 66 + 66],
                            bv_sb[:, HD:HD + 2],
                        )

        if stop_after == "C":
            for t in range(2):
                nc.sync.dma_start(yt[t * 128:(t + 1) * 128, :], qt_sb[t][:].bitcast(f32))
                nc.sync.dma_start(yt[256 + t * 128:256 + (t + 1) * 128, :], kt_sb[t][:].bitcast(f32))
            for lb in range(4):
                nc.sync.dma_start(
                    yt[512 + lb * 128: 512 + (lb + 1) * 128, 0:HPC * 66],
                    vaug_sb[lb][:].bitcast(f32),
                )
        # ---- phase D: attention ----
        run_d = stop_after not in ("C", "noD")
        with tc.tile_pool(name="psp", bufs=2, space="PSUM") as psp, \
             tc.tile_pool(name="pop", bufs=1, space="PSUM") as pop, \
             tc.tile_pool(name="ptp", bufs=3) as ptp, \
             tc.tile_pool(name="rcp", bufs=3) as rcp:
            for h in range(HPC if run_d else 0):
                t, po = h // 2, (h % 2) * 64
                # 4 persistent accumulators, one per q-chunk (4 psum banks)
                pouts = [pop.tile([66, QCW], f32, tag=f"pout{qc}", name=f"pout{qc}")
                         for qc in range(NQC)]
                for kb in range(NKB):
                    for qp in range(2):  # q-chunk pairs -> [128,1024] psum tiles
                        ps = psp.tile([128, 2 * QCW], f32, tag="ps", name="ps")
                        for j in range(2):
                            qc = qp * 2 + j
                            nc.tensor.matmul(
                                ps[:, j * QCW:(j + 1) * QCW],
                                kt_sb[t][po:po + 64, kb * KBW:(kb + 1) * KBW],
                                qt_sb[t][po:po + 64, qc * QCW:(qc + 1) * QCW],
                                start=True, stop=True,
                            )
                        pt = ptp.tile([128, 2 * QCW], f32r, tag="pt", name="pt")
                        nc.scalar.activation(pt[:], ps[:], AF.Exp, scale=SCALE)
                        for j in range(2):
                            qc = qp * 2 + j
                            delta = kb * KBW - qc * QCW
                            if delta in slots:
                                si, c0, c1 = slots[delta]
                                nc.vector.tensor_mul(
                                    pt[:, j * QCW + c0: j * QCW + c1],
                                    pt[:, j * QCW + c0: j * QCW + c1],
                                    em_sb[:, si * 512 + c0: si * 512 + c1],
                                )
                        for j in range(2):
                            qc = qp * 2 + j
                            nc.tensor.matmul(
                                pouts[qc][:],
                                vaug_sb[kb][:, h * 66:(h + 1) * 66],
                                pt[:, j * QCW:(j + 1) * QCW],
                                start=(kb == 0), stop=(kb == NKB - 1),
                            )
                for qc in range(NQC):
                    # rows 0..63 are outT, row 64 is the softmax sum; copy to
                    # SBUF immediately so the next head's kb loop gets the bank
                    po_sb = rcp.tile([66, QCW], f32, tag="po_sb", name="po_sb")
                    nc.scalar.copy(po_sb[:], pouts[qc][:])
                    rec32 = rcp.tile([1, QCW], f32, tag="rec32", name="rec32")
                    nc.vector.reciprocal(rec32[:], po_sb[64:65, :])
                    rec = rcp.tile([1, QCW], f32r, tag="rec", name="rec")
                    nc.vector.tensor_copy(rec[:], rec32[:])
                    pbt = psp.tile([128, 2 * QCW], f32, tag="ps", name="pbt")
                    pb = pbt[0:64, 0:QCW]
                    nc.tensor.matmul(pb, on1_sb[:], rec[:], start=True, stop=True)
                    bc = rcp.tile([64, QCW], f32, tag="bc", name="bc")
                    nc.vector.tensor_copy(bc[:], pb)
                    nc.vector.tensor_mul(
                        ot_sb[h][:, qc * QCW:(qc + 1) * QCW],
                        po_sb[0:64, :], bc[:],
                    )

        if stop_after == "D":
            for h in range(HPC):
                nc.sync.dma_start(yt[h * 64:(h + 1) * 64, :], ot_sb[h][:].bitcast(f32))
        run_e = stop_after in (None, "noD")
        if not run_d and run_e:
            for h in range(HPC):
                nc.sync.dma_start(ot_sb[h][:], xq[h * 64:h * 64 + 64, :].bitcast(f32r))
        # ---- phase E: output projection yT partial ----
        with tc.tile_pool(name="ysp", bufs=3) as ysp, \
             tc.tile_pool(name="pyp", bufs=2, space="PSUM") as pyp:
            for db in range(NDC if run_e else 0):
                py = pyp.tile([128, NQC * QCW], f32, tag="py", name="py")  # 4 banks
                for h in range(HPC):
                    for qc in range(NQC):
                        nc.tensor.matmul(
                            py[:, qc * QCW:(qc + 1) * QCW],
                            wo_sb[:, h * D + db * DCW: h * D + (db + 1) * DCW],
                            ot_sb[h][:, qc * QCW:(qc + 1) * QCW],
                            start=(h == 0), stop=(h == HPC - 1),
                        )
                y_sb = ysp.tile([128, NQC * QCW], f32, tag="y", name="y")
                nc.vector.tensor_copy(y_sb[:], py[:])
                nc.sync.dma_start(yt[db * DCW:(db + 1) * DCW, :], y_sb[:])

    nc.compile()
    return nc


def _pack_ndc(w_g):
    """[HD, D] row-slice of a Linear weight -> [128, NDC*HD] SBUF image with
    w[p, c*HD+n] = w_g[n, c*128+p] (lhsT chunks along the free dim)."""
    return np.ascontiguousarray(
        w_g.reshape(HD, NDC, 128).transpose(2, 1, 0).reshape(128, NDC * HD)
    )


def _prep_inmaps(query, key, value, Wq, bq, Wk, bk, Wv, bv, Wo, masksize):
    half = int(masksize) // 2
    slots = _band_slots(half)
    ns = max(len(slots), 1)
    em = np.ones((128, ns * 512), np.float32)
    e1 = np.float32(np.exp(np.float32(1.0)))
    p = np.arange(128)[:, None]
    f = np.arange(512)[None, :]
    for d, (si, _, _) in slots.items():
        em[:, si * 512:(si + 1) * 512] = np.where(
            np.abs(d + p - f) <= half, e1, np.float32(1.0)
        )

    xqT = [np.ascontiguousarray(query[b].T) for b in range(B)]
    xkT = [np.ascontiguousarray(key[b].T) for b in range(B)]
    xvT = [np.ascontiguousarray(value[b].T) for b in range(B)]
    wqP = [_pack_ndc(Wq[g * HD:(g + 1) * HD, :]) for g in range(4)]
    wkP = [_pack_ndc(Wk[g * HD:(g + 1) * HD, :]) for g in range(4)]
    wvP = [_pack_ndc(Wv[g * HD:(g + 1) * HD, :]) for g in range(4)]
    # wo[p, h*D+n] = Wo[n, g*HD + h*64 + p]  (per-head base-0 lhsT blocks)
    woP = [
        np.ascontiguousarray(
            Wo[:, g * HD:(g + 1) * HD].reshape(D, HPC, 64).transpose(2, 1, 0).reshape(64, HPC * D)
        )
        for g in range(4)
    ]
    bqP = [np.ascontiguousarray(bq[g * HD:(g + 1) * HD].reshape(2, 128).T) for g in range(4)]
    bkP = [np.ascontiguousarray(bk[g * HD:(g + 1) * HD].reshape(2, 128).T) for g in range(4)]
    bvP = [
        np.ascontiguousarray(
            np.concatenate(
                [np.tile(bv[g * HD:(g + 1) * HD], (128, 1)), np.ones((128, 2), np.float32)],
                axis=1,
            )
        )
        for g in range(4)
    ]

    in_maps = []
    for c in range(8):
        b, g = c // 4, c % 4
        in_maps.append({
            "xq": xqT[b], "xk": xkT[b], "xv": xvT[b],
            "wq": wqP[g], "wk": wkP[g], "wv": wvP[g], "wo": woP[g],
            "bq": bqP[g], "bk": bkP[g], "bv": bvP[g], "em": em,
            "on1": np.ones((1, 64), np.float32),
        })
    return in_maps


def kernel(query, key, value, Wq, bq, Wk, bk, Wv, bv, Wo, bo, masksize):
    query = np.asarray(query, dtype=np.float32)
    key = np.asarray(key, dtype=np.float32)
    value = np.asarray(value, dtype=np.float32)
    Wq, bq = np.asarray(Wq, np.float32), np.asarray(bq, np.float32)
    Wk, bk = np.asarray(Wk, np.float32), np.asarray(bk, np.float32)
    Wv, bv = np.asarray(Wv, np.float32), np.asarray(bv, np.float32)
    Wo, bo = np.asarray(Wo, np.float32), np.asarray(bo, np.float32)
    ms = int(np.asarray(masksize))

    if ms not in _CACHE:
        _CACHE[ms] = _build(ms)
    nc = _CACHE[ms]

    in_maps = _prep_inmaps(query, key, value, Wq, bq, Wk, bk, Wv, bv, Wo, ms)
    res = run_bass_kernel_spmd(nc, in_maps, list(range(8)))

    out = np.empty((B, L, D), np.float32)
    for b in range(B):
        acc = res.results[4 * b]["yt"].astype(np.float32)
        for g in range(1, 4):
            acc = acc + res.results[4 * b + g]["yt"]
        out[b] = acc.T + bo
    return out



# revision 3
# speedup vs baseline: 1.2821x; 1.2821x over previous
"""Banded multi-head attention on 8 Trainium2 NeuronCores.

Problem: B=2, L=2048, D=1024, H=16 heads, d_k=64. The band mask is a 0/1
FLOAT tensor *added* to the scores (not -inf masked), so the softmax is
dense; exp(s + band) = exp(s) * e^band is handled by multiplying constant
e-or-1 parallelogram tiles over the band region.

Sharding: (batch x 4-head-groups) across the 8 cores. Host pre-transposes
activations/weights so every device matmul has its contraction dim on
partitions; the per-core partial output projections are summed on the host
(gather/unshard) together with the output bias.

All device matmuls run in bf16 (fast-weight-load PE path); accumulation is
fp32 in PSUM. Softmax normalization is a batched approx-reciprocal on DVE
plus a gpsimd partition-broadcast, keeping the scalar engine free for the
exp() stream that bounds phase D.
"""

import sys

sys.path.insert(0, "/opt/trn_rl_repo")

import numpy as np
import ml_dtypes
from contextlib import ExitStack

import concourse.bass as bass
import concourse.tile as tile
from concourse import bacc, mybir
from concourse.bass_utils import run_bass_kernel_spmd

dt = mybir.dt
AF = mybir.ActivationFunctionType
bf16 = dt.bfloat16
nbf16 = ml_dtypes.bfloat16

B, L, D, H, DK = 2, 2048, 1024, 16, 64
HPC = 4            # heads per core
HD = HPC * DK      # 256: head dims per core
NQC, QCW = 4, 512  # q chunks
NKB, KBW = 16, 128 # k blocks
NDC, DCW = 8, 128  # D chunks
SCALE = 1.0 / 8.0  # 1/sqrt(d_k)

_CACHE = {}


def _band_slots(half):
    """delta -> (slot, c0, c1) for 128x512 tiles at k-offset kb*128, q-offset
    qc*512, delta = kb*128 - qc*512. Band cols: f in [delta-half, delta+127+half]."""
    slots = {}
    d = -((half + 127) // 128) * 128
    while d <= half + 511:
        c0, c1 = max(0, d - half), min(512, d + 128 + half)
        if c0 < c1:
            slots[d] = (len(slots), c0, c1)
        d += 128
    return slots


def _build(masksize):
    half = int(masksize) // 2
    slots = _band_slots(half)
    ns = max(len(slots), 1)

    nc = bacc.Bacc("TRN2", target_bir_lowering=False, debug=False)

    f32 = dt.float32
    xq = nc.dram_tensor("xq", [D, L], bf16, kind="ExternalInput").ap()
    xk = nc.dram_tensor("xk", [D, L], bf16, kind="ExternalInput").ap()
    xv = nc.dram_tensor("xv", [D, L], bf16, kind="ExternalInput").ap()
    # weights pre-packed on host into SBUF layouts (see _prep_inmaps)
    wq = nc.dram_tensor("wq", [128, NDC * HD], bf16, kind="ExternalInput").ap()
    wk = nc.dram_tensor("wk", [128, NDC * HD], bf16, kind="ExternalInput").ap()
    wv = nc.dram_tensor("wv", [128, NDC * HD], bf16, kind="ExternalInput").ap()
    wo = nc.dram_tensor("wo", [64, HPC * D], bf16, kind="ExternalInput").ap()
    bq = nc.dram_tensor("bq", [128, 2], f32, kind="ExternalInput").ap()
    bk = nc.dram_tensor("bk", [128, 2], f32, kind="ExternalInput").ap()
    bv = nc.dram_tensor("bv", [128, HD + 2], f32, kind="ExternalInput").ap()
    em = nc.dram_tensor("em", [128, ns * 512], bf16, kind="ExternalInput").ap()
    yt = nc.dram_tensor("yt", [D, L], f32, kind="ExternalOutput").ap()

    with tile.TileContext(nc) as tc, ExitStack() as ctx:
        ctx.enter_context(
            nc.allow_low_precision(reason="bf16 matmul operands are intentional")
        )
        # ---- persistent SBUF ----
        wts = ctx.enter_context(tc.tile_pool(name="wts", bufs=1))
        big = ctx.enter_context(tc.tile_pool(name="big", bufs=1))

        wq_sb = wts.tile([128, NDC * HD], bf16, tag="wq", name="wq")
        wk_sb = wts.tile([128, NDC * HD], bf16, tag="wk", name="wk")
        bq_sb = wts.tile([128, 2], f32, tag="bq", name="bq")
        bk_sb = wts.tile([128, 2], f32, tag="bk", name="bk")
        # phase B needs these first: issue on the sync queue ahead of x tiles
        for t_sb, t_in in ((wq_sb, wq), (bq_sb, bq), (wk_sb, wk), (bk_sb, bk)):
            nc.sync.dma_start(t_sb[:], t_in[:])
        # phase C/D/E weights: separate queue (gpsimd) so they don't delay B
        wv_sb = wts.tile([128, NDC * HD], bf16, tag="wv", name="wv")
        wo_sb = wts.tile([64, HPC * D], bf16, tag="wo", name="wo")
        bv_sb = wts.tile([128, HD + 2], f32, tag="bv", name="bv")
        em_sb = wts.tile([128, ns * 512], bf16, tag="em", name="em")
        for t_sb, t_in in ((wv_sb, wv), (bv_sb, bv), (em_sb, em), (wo_sb, wo)):
            nc.gpsimd.dma_start(t_sb[:], t_in[:])

        # projection outputs (resident)
        qt_sb = [big.tile([128, L], bf16, tag=f"qt{t}", name=f"qt{t}") for t in range(2)]
        kt_sb = [big.tile([128, L], bf16, tag=f"kt{t}", name=f"kt{t}") for t in range(2)]
        ot_sb = [big.tile([64, L], bf16, tag=f"ot{h}", name=f"ot{h}") for h in range(HPC)]
        # v (natural layout) + ones col per head: [128, HPC*66] per k-block
        vaug_sb = [big.tile([128, HPC * 66], bf16, tag=f"vaug{lb}", name=f"vaug{lb}") for lb in range(NKB)]

        # ---- phase B: q/k projections (T-layout) ----
        with tc.tile_pool(name="xs", bufs=3) as xs, \
             tc.tile_pool(name="pqk", bufs=2, space="PSUM") as pqk:
            for qc in range(NQC):
                pq = [pqk.tile([128, QCW], f32, tag=f"pq{t}", name=f"pq{t}") for t in range(2)]
                pk = [pqk.tile([128, QCW], f32, tag=f"pk{t}", name=f"pk{t}") for t in range(2)]
                for c in range(NDC):
                    xq_t = xs.tile([128, QCW], bf16, tag="xq", name="xq")
                    nc.sync.dma_start(
                        xq_t[:], xq[c * DCW:(c + 1) * DCW, qc * QCW:(qc + 1) * QCW]
                    )
                    xk_t = xs.tile([128, QCW], bf16, tag="xk", name="xk")
                    nc.sync.dma_start(
                        xk_t[:], xk[c * DCW:(c + 1) * DCW, qc * QCW:(qc + 1) * QCW]
                    )
                    for t in range(2):
                        nc.tensor.matmul(
                            pq[t][:], wq_sb[:, c * HD + t * 128: c * HD + (t + 1) * 128],
                            xq_t[:], start=(c == 0), stop=(c == NDC - 1),
                        )
                        nc.tensor.matmul(
                            pk[t][:], wk_sb[:, c * HD + t * 128: c * HD + (t + 1) * 128],
                            xk_t[:], start=(c == 0), stop=(c == NDC - 1),
                        )
                for t in range(2):
                    nc.scalar.activation(
                        qt_sb[t][:, qc * QCW:(qc + 1) * QCW], pq[t][:],
                        AF.Identity, bias=bq_sb[:, t:t + 1],
                    )
                    nc.scalar.activation(
                        kt_sb[t][:, qc * QCW:(qc + 1) * QCW], pk[t][:],
                        AF.Identity, bias=bk_sb[:, t:t + 1],
                    )

        # ---- phase C: v in natural [L, HD] layout, +bias, +ones col ----
        with tc.tile_pool(name="xvp", bufs=3) as xvs, \
             tc.tile_pool(name="pvp", bufs=2, space="PSUM") as pvp:
            for lg in range(4):  # groups of 4 k-blocks
                pv = [pvp.tile([128, HD], f32, tag=f"pv{j}", name=f"pv{j}") for j in range(4)]
                for c in range(NDC):
                    xv_t = xvs.tile([128, QCW], bf16, tag="xv", name="xv")
                    nc.sync.dma_start(
                        xv_t[:], xv[c * DCW:(c + 1) * DCW, lg * QCW:(lg + 1) * QCW]
                    )
                    for j in range(4):
                        nc.tensor.matmul(
                            pv[j][:], xv_t[:, j * 128:(j + 1) * 128],
                            wv_sb[:, c * HD:(c + 1) * HD],
                            start=(c == 0), stop=(c == NDC - 1),
                        )
                for j in range(4):
                    lb = lg * 4 + j
                    for h in range(HPC):
                        nc.vector.tensor_add(
                            vaug_sb[lb][:, h * 66: h * 66 + 64],
                            pv[j][:, h * DK:(h + 1) * DK],
                            bv_sb[:, h * DK:(h + 1) * DK],
                        )
                        nc.vector.tensor_copy(
                            vaug_sb[lb][:, h * 66 + 64: h * 66 + 66],
                            bv_sb[:, HD:HD + 2],
                        )

        # ---- phase D: attention ----
        with tc.tile_pool(name="psp", bufs=2, space="PSUM") as psp, \
             tc.tile_pool(name="pop", bufs=2, space="PSUM") as pop, \
             tc.tile_pool(name="ptp", bufs=3) as ptp, \
             tc.tile_pool(name="rcp", bufs=2) as rcp:
            for h in range(HPC):
                t, po = h // 2, (h % 2) * 64
                for qp in range(2):  # pairs of q chunks
                    pout = pop.tile([66, 2 * QCW], f32, tag="pout", name="pout")
                    for kb in range(NKB):
                        ps = psp.tile([128, 2 * QCW], f32, tag="ps", name="ps")
                        for j in range(2):
                            qc = qp * 2 + j
                            nc.tensor.matmul(
                                ps[:, j * QCW:(j + 1) * QCW],
                                kt_sb[t][po:po + 64, kb * KBW:(kb + 1) * KBW],
                                qt_sb[t][po:po + 64, qc * QCW:(qc + 1) * QCW],
                                start=True, stop=True,
                            )
                        pt = ptp.tile([128, 2 * QCW], bf16, tag="pt", name="pt")
                        nc.scalar.activation(pt[:], ps[:], AF.Exp, scale=SCALE)
                        for j in range(2):
                            qc = qp * 2 + j
                            delta = kb * KBW - qc * QCW
                            if delta in slots:
                                si, c0, c1 = slots[delta]
                                nc.vector.tensor_mul(
                                    pt[:, j * QCW + c0: j * QCW + c1],
                                    pt[:, j * QCW + c0: j * QCW + c1],
                                    em_sb[:, si * 512 + c0: si * 512 + c1],
                                )
                        for j in range(2):
                            nc.tensor.matmul(
                                pout[:, j * QCW:(j + 1) * QCW],
                                vaug_sb[kb][:, h * 66:(h + 1) * 66],
                                pt[:, j * QCW:(j + 1) * QCW],
                                start=(kb == 0), stop=(kb == NKB - 1),
                            )
                    # normalize: rows 0..63 are outT, row 64 is the softmax sum
                    # (copy the sum row to partition 0 first: the custom-DVE
                    # reciprocal does not honor partition offsets)
                    srow = rcp.tile([1, 2 * QCW], f32, tag="srow", name="srow")
                    nc.vector.tensor_copy(srow[:], pout[64:65, :])
                    rec = rcp.tile([1, 2 * QCW], f32, tag="rec", name="rec")
                    nc.vector.reciprocal_approx_fast(rec[:], srow[:])
                    bc = rcp.tile([64, 2 * QCW], f32, tag="bc", name="bc")
                    nc.gpsimd.partition_broadcast(bc[:], rec[:])
                    for j in range(2):
                        qc = qp * 2 + j
                        nc.vector.tensor_mul(
                            ot_sb[h][:, qc * QCW:(qc + 1) * QCW],
                            pout[0:64, j * QCW:(j + 1) * QCW],
                            bc[:, j * QCW:(j + 1) * QCW],
                        )

        # ---- phase E: output projection yT partial ----
        with tc.tile_pool(name="ysp", bufs=2) as ysp, \
             tc.tile_pool(name="pyp", bufs=2, space="PSUM") as pyp:
            for db in range(NDC):
                py = pyp.tile([128, NQC * QCW], f32, tag="py", name="py")  # 4 banks
                for h in range(HPC):
                    for qc in range(NQC):
                        nc.tensor.matmul(
                            py[:, qc * QCW:(qc + 1) * QCW],
                            wo_sb[:, h * D + db * DCW: h * D + (db + 1) * DCW],
                            ot_sb[h][:, qc * QCW:(qc + 1) * QCW],
                            start=(h == 0), stop=(h == HPC - 1),
                        )
                y_sb = ysp.tile([128, NQC * QCW], f32, tag="y", name="y")
                if db % 2 == 0:
                    nc.scalar.copy(y_sb[:], py[:])
                else:
                    nc.vector.tensor_copy(y_sb[:], py[:])
                nc.sync.dma_start(yt[db * DCW:(db + 1) * DCW, :], y_sb[:])

    nc.compile()
    return nc


def _pack_ndc(w_g):
    """[HD, D] row-slice of a Linear weight -> [128, NDC*HD] SBUF image with
    w[p, c*HD+n] = w_g[n, c*128+p] (lhsT chunks along the free dim)."""
    return np.ascontiguousarray(
        w_g.reshape(HD, NDC, 128).transpose(2, 1, 0).reshape(128, NDC * HD)
    )


def _prep_inmaps(query, key, value, Wq, bq, Wk, bk, Wv, bv, Wo, masksize):
    half = int(masksize) // 2
    slots = _band_slots(half)
    ns = max(len(slots), 1)
    em = np.ones((128, ns * 512), np.float32)
    e1 = np.float32(np.exp(np.float32(1.0)))
    p = np.arange(128)[:, None]
    f = np.arange(512)[None, :]
    for d, (si, _, _) in slots.items():
        em[:, si * 512:(si + 1) * 512] = np.where(
            np.abs(d + p - f) <= half, e1, np.float32(1.0)
        )
    em = em.astype(nbf16)

    xqT = [np.ascontiguousarray(query[b].T).astype(nbf16) for b in range(B)]
    xkT = [np.ascontiguousarray(key[b].T).astype(nbf16) for b in range(B)]
    xvT = [np.ascontiguousarray(value[b].T).astype(nbf16) for b in range(B)]
    wqP = [_pack_ndc(Wq[g * HD:(g + 1) * HD, :]).astype(nbf16) for g in range(4)]
    wkP = [_pack_ndc(Wk[g * HD:(g + 1) * HD, :]).astype(nbf16) for g in range(4)]
    wvP = [_pack_ndc(Wv[g * HD:(g + 1) * HD, :]).astype(nbf16) for g in range(4)]
    # wo[p, h*D+n] = Wo[n, g*HD + h*64 + p]  (per-head base-0 lhsT blocks)
    woP = [
        np.ascontiguousarray(
            Wo[:, g * HD:(g + 1) * HD].reshape(D, HPC, 64).transpose(2, 1, 0).reshape(64, HPC * D)
        ).astype(nbf16)
        for g in range(4)
    ]
    bqP = [np.ascontiguousarray(bq[g * HD:(g + 1) * HD].reshape(2, 128).T) for g in range(4)]
    bkP = [np.ascontiguousarray(bk[g * HD:(g + 1) * HD].reshape(2, 128).T) for g in range(4)]
    bvP = [
        np.ascontiguousarray(
            np.concatenate(
                [np.tile(bv[g * HD:(g + 1) * HD], (128, 1)), np.ones((128, 2), np.float32)],
                axis=1,
            )
        )
        for g in range(4)
    ]

    in_maps = []
    for c in range(8):
        b, g = c // 4, c % 4
        in_maps.append({
            "xq": xqT[b], "xk": xkT[b], "xv": xvT[b],
            "wq": wqP[g], "wk": wkP[g], "wv": wvP[g], "wo": woP[g],
            "bq": bqP[g], "bk": bkP[g], "bv": bvP[g], "em": em,
        })
    return in_maps


def kernel(query, key, value, Wq, bq, Wk, bk, Wv, bv, Wo, bo, masksize):
    query = np.asarray(query, dtype=np.float32)
    key = np.asarray(key, dtype=np.float32)
    value = np.asarray(value, dtype=np.float32)
    Wq, bq = np.asarray(Wq, np.float32), np.asarray(bq, np.float32)
    Wk, bk = np.asarray(Wk, np.float32), np.asarray(bk, np.float32)
    Wv, bv = np.asarray(Wv, np.float32), np.asarray(bv, np.float32)
    Wo, bo = np.asarray(Wo, np.float32), np.asarray(bo, np.float32)
    ms = int(np.asarray(masksize))

    if ms not in _CACHE:
        _CACHE[ms] = _build(ms)
    nc = _CACHE[ms]

    in_maps = _prep_inmaps(query, key, value, Wq, bq, Wk, bk, Wv, bv, Wo, ms)
    res = run_bass_kernel_spmd(nc, in_maps, list(range(8)))

    out = np.empty((B, L, D), np.float32)
    for b in range(B):
        acc = res.results[4 * b]["yt"].astype(np.float32)
        for g in range(1, 4):
            acc = acc + res.results[4 * b + g]["yt"]
        out[b] = acc.T + bo
    return out


# revision 4
# speedup vs baseline: 1.5749x; 1.2284x over previous
"""Banded multi-head attention on 8 Trainium2 NeuronCores.

Problem: B=2, L=2048, D=1024, H=16 heads, d_k=64. The band mask is a 0/1
FLOAT tensor *added* to the scores (not -inf masked), so the softmax is
dense; exp(s + band) = exp(s) * e^band is handled by multiplying constant
e-or-1 parallelogram tiles over the band region.

Sharding: (batch x 4-head-groups) across the 8 cores. Host pre-transposes
activations/weights so every device matmul has its contraction dim on
partitions; the per-core partial output projections are summed on the host
(gather/unshard) together with the output bias.

All device matmuls run in bf16 (fast-weight-load PE path); accumulation is
fp32 in PSUM. Softmax normalization is a batched approx-reciprocal on DVE
plus a gpsimd partition-broadcast, keeping the scalar engine free for the
exp() stream that bounds phase D.
"""

import sys

sys.path.insert(0, "/opt/trn_rl_repo")

import numpy as np
import ml_dtypes
from contextlib import ExitStack

import concourse.bass as bass
import concourse.tile as tile
from concourse import bacc, mybir
from concourse.bass_utils import run_bass_kernel_spmd

dt = mybir.dt
AF = mybir.ActivationFunctionType
bf16 = dt.bfloat16
nbf16 = ml_dtypes.bfloat16

B, L, D, H, DK = 2, 2048, 1024, 16, 64
HPC = 4            # heads per core
HD = HPC * DK      # 256: head dims per core
NQC, QCW = 4, 512  # q chunks
NKB, KBW = 16, 128 # k blocks
NDC, DCW = 8, 128  # D chunks
SCALE = 1.0 / 8.0  # 1/sqrt(d_k)

_CACHE = {}


def _band_slots(half):
    """delta -> (slot, c0, c1) for 128x512 tiles at k-offset kb*128, q-offset
    qc*512, delta = kb*128 - qc*512. Band cols: f in [delta-half, delta+127+half]."""
    slots = {}
    d = -((half + 127) // 128) * 128
    while d <= half + 511:
        c0, c1 = max(0, d - half), min(512, d + 128 + half)
        if c0 < c1:
            slots[d] = (len(slots), c0, c1)
        d += 128
    return slots


def _build(masksize):
    half = int(masksize) // 2
    slots = _band_slots(half)
    ns = max(len(slots), 1)

    nc = bacc.Bacc("TRN2", target_bir_lowering=False, debug=False)

    f32 = dt.float32
    xq = nc.dram_tensor("xq", [D, L], bf16, kind="ExternalInput").ap()
    xk = nc.dram_tensor("xk", [D, L], bf16, kind="ExternalInput").ap()
    xv = nc.dram_tensor("xv", [D, L], bf16, kind="ExternalInput").ap()
    # weights pre-packed on host into SBUF layouts (see _prep_inmaps)
    wq = nc.dram_tensor("wq", [128, NDC * HD], bf16, kind="ExternalInput").ap()
    wk = nc.dram_tensor("wk", [128, NDC * HD], bf16, kind="ExternalInput").ap()
    wv = nc.dram_tensor("wv", [128, NDC * HD], bf16, kind="ExternalInput").ap()
    wo = nc.dram_tensor("wo", [64, HPC * D], bf16, kind="ExternalInput").ap()
    bq = nc.dram_tensor("bq", [128, 2], f32, kind="ExternalInput").ap()
    bk = nc.dram_tensor("bk", [128, 2], f32, kind="ExternalInput").ap()
    bv = nc.dram_tensor("bv", [128, HD + 2], f32, kind="ExternalInput").ap()
    em = nc.dram_tensor("em", [128, ns * 512], bf16, kind="ExternalInput").ap()
    yt = nc.dram_tensor("yt", [D, L], f32, kind="ExternalOutput").ap()

    with tile.TileContext(nc) as tc, ExitStack() as ctx:
        ctx.enter_context(
            nc.allow_low_precision(reason="bf16 matmul operands are intentional")
        )
        # ---- persistent SBUF ----
        wts = ctx.enter_context(tc.tile_pool(name="wts", bufs=1))
        big = ctx.enter_context(tc.tile_pool(name="big", bufs=1))

        wq_sb = wts.tile([128, NDC * HD], bf16, tag="wq", name="wq")
        wk_sb = wts.tile([128, NDC * HD], bf16, tag="wk", name="wk")
        bq_sb = wts.tile([128, 2], f32, tag="bq", name="bq")
        bk_sb = wts.tile([128, 2], f32, tag="bk", name="bk")
        # phase B needs these first: issue on the sync queue ahead of x tiles
        for t_sb, t_in in ((wq_sb, wq), (bq_sb, bq), (wk_sb, wk), (bk_sb, bk)):
            nc.sync.dma_start(t_sb[:], t_in[:])
        # phase C/D/E weights: separate queue (gpsimd) so they don't delay B
        wv_sb = wts.tile([128, NDC * HD], bf16, tag="wv", name="wv")
        wo_sb = wts.tile([64, HPC * D], bf16, tag="wo", name="wo")
        bv_sb = wts.tile([128, HD + 2], f32, tag="bv", name="bv")
        em_sb = wts.tile([128, ns * 512], bf16, tag="em", name="em")
        for t_sb, t_in in ((wv_sb, wv), (bv_sb, bv), (em_sb, em), (wo_sb, wo)):
            nc.gpsimd.dma_start(t_sb[:], t_in[:])

        # projection outputs (resident)
        qt_sb = [big.tile([128, L], bf16, tag=f"qt{t}", name=f"qt{t}") for t in range(2)]
        kt_sb = [big.tile([128, L], bf16, tag=f"kt{t}", name=f"kt{t}") for t in range(2)]
        ot_sb = [big.tile([64, L], bf16, tag=f"ot{h}", name=f"ot{h}") for h in range(HPC)]
        # v (natural layout) + ones col per head: [128, HPC*66] per k-block
        vaug_sb = [big.tile([128, HPC * 66], bf16, tag=f"vaug{lb}", name=f"vaug{lb}") for lb in range(NKB)]

        # ---- phase B: q/k projections (T-layout) ----
        with tc.tile_pool(name="xs", bufs=3) as xs, \
             tc.tile_pool(name="pqk", bufs=2, space="PSUM") as pqk:
            for qc in range(NQC):
                pq = [pqk.tile([128, QCW], f32, tag=f"pq{t}", name=f"pq{t}") for t in range(2)]
                pk = [pqk.tile([128, QCW], f32, tag=f"pk{t}", name=f"pk{t}") for t in range(2)]
                for c in range(NDC):
                    xq_t = xs.tile([128, QCW], bf16, tag="xq", name="xq")
                    nc.sync.dma_start(
                        xq_t[:], xq[c * DCW:(c + 1) * DCW, qc * QCW:(qc + 1) * QCW]
                    )
                    xk_t = xs.tile([128, QCW], bf16, tag="xk", name="xk")
                    nc.sync.dma_start(
                        xk_t[:], xk[c * DCW:(c + 1) * DCW, qc * QCW:(qc + 1) * QCW]
                    )
                    for t in range(2):
                        nc.tensor.matmul(
                            pq[t][:], wq_sb[:, c * HD + t * 128: c * HD + (t + 1) * 128],
                            xq_t[:], start=(c == 0), stop=(c == NDC - 1),
                        )
                        nc.tensor.matmul(
                            pk[t][:], wk_sb[:, c * HD + t * 128: c * HD + (t + 1) * 128],
                            xk_t[:], start=(c == 0), stop=(c == NDC - 1),
                        )
                for t in range(2):
                    nc.scalar.activation(
                        qt_sb[t][:, qc * QCW:(qc + 1) * QCW], pq[t][:],
                        AF.Identity, bias=bq_sb[:, t:t + 1],
                    )
                    nc.scalar.activation(
                        kt_sb[t][:, qc * QCW:(qc + 1) * QCW], pk[t][:],
                        AF.Identity, bias=bk_sb[:, t:t + 1],
                    )

        # ---- phase C: v in natural [L, HD] layout, +bias, +ones col ----
        with tc.tile_pool(name="xvp", bufs=3) as xvs, \
             tc.tile_pool(name="pvp", bufs=2, space="PSUM") as pvp:
            for lg in range(4):  # groups of 4 k-blocks
                pv = [pvp.tile([128, HD], f32, tag=f"pv{j}", name=f"pv{j}") for j in range(4)]
                for c in range(NDC):
                    xv_t = xvs.tile([128, QCW], bf16, tag="xv", name="xv")
                    nc.sync.dma_start(
                        xv_t[:], xv[c * DCW:(c + 1) * DCW, lg * QCW:(lg + 1) * QCW]
                    )
                    for j in range(4):
                        nc.tensor.matmul(
                            pv[j][:], xv_t[:, j * 128:(j + 1) * 128],
                            wv_sb[:, c * HD:(c + 1) * HD],
                            start=(c == 0), stop=(c == NDC - 1),
                        )
                for j in range(4):
                    lb = lg * 4 + j
                    for h in range(HPC):
                        nc.vector.tensor_add(
                            vaug_sb[lb][:, h * 66: h * 66 + 64],
                            pv[j][:, h * DK:(h + 1) * DK],
                            bv_sb[:, h * DK:(h + 1) * DK],
                        )
                        nc.vector.tensor_copy(
                            vaug_sb[lb][:, h * 66 + 64: h * 66 + 66],
                            bv_sb[:, HD:HD + 2],
                        )

        # ---- phase D: attention ----
        # Two dense PE stretches per (head, q-pair) unit: 32 score matmuls
        # (exp trailing on ACT into parked SBUF pt tiles), then 32 PV
        # matmuls. This keeps the PE stream gap-free so the HAM activity
        # monitor holds the full-speed grant; the per-kb serial chain
        # (scores->exp->band->PV) would otherwise micro-stall it into the
        # half-duty throttle state.
        with tc.tile_pool(name="psp", bufs=2, space="PSUM") as psp, \
             tc.tile_pool(name="pop", bufs=2, space="PSUM") as pop, \
             tc.tile_pool(name="ptp", bufs=NKB + 4) as ptp, \
             tc.tile_pool(name="rcp", bufs=2) as rcp:
            for h in range(HPC):
                t, po = h // 2, (h % 2) * 64
                for qp in range(2):  # pairs of q chunks
                    pts = []
                    for kb in range(NKB):
                        ps = psp.tile([128, 2 * QCW], f32, tag="ps", name="ps")
                        for j in range(2):
                            qc = qp * 2 + j
                            nc.tensor.matmul(
                                ps[:, j * QCW:(j + 1) * QCW],
                                kt_sb[t][po:po + 64, kb * KBW:(kb + 1) * KBW],
                                qt_sb[t][po:po + 64, qc * QCW:(qc + 1) * QCW],
                                start=True, stop=True,
                            )
                        pt = ptp.tile([128, 2 * QCW], bf16, tag="pt", name="pt")
                        nc.scalar.activation(pt[:], ps[:], AF.Exp, scale=SCALE)
                        for j in range(2):
                            qc = qp * 2 + j
                            delta = kb * KBW - qc * QCW
                            if delta in slots:
                                si, c0, c1 = slots[delta]
                                nc.vector.tensor_mul(
                                    pt[:, j * QCW + c0: j * QCW + c1],
                                    pt[:, j * QCW + c0: j * QCW + c1],
                                    em_sb[:, si * 512 + c0: si * 512 + c1],
                                )
                        pts.append(pt)
                    pout = pop.tile([66, 2 * QCW], f32, tag="pout", name="pout")
                    for kb in range(NKB):
                        for j in range(2):
                            nc.tensor.matmul(
                                pout[:, j * QCW:(j + 1) * QCW],
                                vaug_sb[kb][:, h * 66:(h + 1) * 66],
                                pts[kb][:, j * QCW:(j + 1) * QCW],
                                start=(kb == 0), stop=(kb == NKB - 1),
                            )
                    # normalize: rows 0..63 are outT, row 64 is the softmax sum
                    # (copy the sum row to partition 0 first: the custom-DVE
                    # reciprocal does not honor partition offsets)
                    srow = rcp.tile([1, 2 * QCW], f32, tag="srow", name="srow")
                    nc.vector.tensor_copy(srow[:], pout[64:65, :])
                    rec = rcp.tile([1, 2 * QCW], f32, tag="rec", name="rec")
                    nc.vector.reciprocal_approx_fast(rec[:], srow[:])
                    bc = rcp.tile([64, 2 * QCW], f32, tag="bc", name="bc")
                    nc.gpsimd.partition_broadcast(bc[:], rec[:])
                    for j in range(2):
                        qc = qp * 2 + j
                        nc.vector.tensor_mul(
                            ot_sb[h][:, qc * QCW:(qc + 1) * QCW],
                            pout[0:64, j * QCW:(j + 1) * QCW],
                            bc[:, j * QCW:(j + 1) * QCW],
                        )

        # ---- phase E: output projection yT partial ----
        with tc.tile_pool(name="ysp", bufs=2) as ysp, \
             tc.tile_pool(name="pyp", bufs=2, space="PSUM") as pyp:
            for db in range(NDC):
                py = pyp.tile([128, NQC * QCW], f32, tag="py", name="py")  # 4 banks
                for h in range(HPC):
                    for qc in range(NQC):
                        nc.tensor.matmul(
                            py[:, qc * QCW:(qc + 1) * QCW],
                            wo_sb[:, h * D + db * DCW: h * D + (db + 1) * DCW],
                            ot_sb[h][:, qc * QCW:(qc + 1) * QCW],
                            start=(h == 0), stop=(h == HPC - 1),
                        )
                y_sb = ysp.tile([128, NQC * QCW], f32, tag="y", name="y")
                if db % 2 == 0:
                    nc.scalar.copy(y_sb[:], py[:])
                else:
                    nc.vector.tensor_copy(y_sb[:], py[:])
                nc.sync.dma_start(yt[db * DCW:(db + 1) * DCW, :], y_sb[:])

    nc.compile()
    return nc


def _pack_ndc(w_g):
    """[HD, D] row-slice of a Linear weight -> [128, NDC*HD] SBUF image with
    w[p, c*HD+n] = w_g[n, c*128+p] (lhsT chunks along the free dim)."""
    return np.ascontiguousarray(
        w_g.reshape(HD, NDC, 128).transpose(2, 1, 0).reshape(128, NDC * HD)
    )


def _prep_inmaps(query, key, value, Wq, bq, Wk, bk, Wv, bv, Wo, masksize):
    half = int(masksize) // 2
    slots = _band_slots(half)
    ns = max(len(slots), 1)
    em = np.ones((128, ns * 512), np.float32)
    e1 = np.float32(np.exp(np.float32(1.0)))
    p = np.arange(128)[:, None]
    f = np.arange(512)[None, :]
    for d, (si, _, _) in slots.items():
        em[:, si * 512:(si + 1) * 512] = np.where(
            np.abs(d + p - f) <= half, e1, np.float32(1.0)
        )
    em = em.astype(nbf16)

    xqT = [np.ascontiguousarray(query[b].T).astype(nbf16) for b in range(B)]
    xkT = [np.ascontiguousarray(key[b].T).astype(nbf16) for b in range(B)]
    xvT = [np.ascontiguousarray(value[b].T).astype(nbf16) for b in range(B)]
    wqP = [_pack_ndc(Wq[g * HD:(g + 1) * HD, :]).astype(nbf16) for g in range(4)]
    wkP = [_pack_ndc(Wk[g * HD:(g + 1) * HD, :]).astype(nbf16) for g in range(4)]
    wvP = [_pack_ndc(Wv[g * HD:(g + 1) * HD, :]).astype(nbf16) for g in range(4)]
    # wo[p, h*D+n] = Wo[n, g*HD + h*64 + p]  (per-head base-0 lhsT blocks)
    woP = [
        np.ascontiguousarray(
            Wo[:, g * HD:(g + 1) * HD].reshape(D, HPC, 64).transpose(2, 1, 0).reshape(64, HPC * D)
        ).astype(nbf16)
        for g in range(4)
    ]
    bqP = [np.ascontiguousarray(bq[g * HD:(g + 1) * HD].reshape(2, 128).T) for g in range(4)]
    bkP = [np.ascontiguousarray(bk[g * HD:(g + 1) * HD].reshape(2, 128).T) for g in range(4)]
    bvP = [
        np.ascontiguousarray(
            np.concatenate(
                [np.tile(bv[g * HD:(g + 1) * HD], (128, 1)), np.ones((128, 2), np.float32)],
                axis=1,
            )
        )
        for g in range(4)
    ]

    in_maps = []
    for c in range(8):
        b, g = c // 4, c % 4
        in_maps.append({
            "xq": xqT[b], "xk": xkT[b], "xv": xvT[b],
            "wq": wqP[g], "wk": wkP[g], "wv": wvP[g], "wo": woP[g],
            "bq": bqP[g], "bk": bkP[g], "bv": bvP[g], "em": em,
        })
    return in_maps


def kernel(query, key, value, Wq, bq, Wk, bk, Wv, bv, Wo, bo, masksize):
    query = np.asarray(query, dtype=np.float32)
    key = np.asarray(key, dtype=np.float32)
    value = np.asarray(value, dtype=np.float32)
    Wq, bq = np.asarray(Wq, np.float32), np.asarray(bq, np.float32)
    Wk, bk = np.asarray(Wk, np.float32), np.asarray(bk, np.float32)
    Wv, bv = np.asarray(Wv, np.float32), np.asarray(bv, np.float32)
    Wo, bo = np.asarray(Wo, np.float32), np.asarray(bo, np.float32)
    ms = int(np.asarray(masksize))

    if ms not in _CACHE:
        _CACHE[ms] = _build(ms)
    nc = _CACHE[ms]

    in_maps = _prep_inmaps(query, key, value, Wq, bq, Wk, bk, Wv, bv, Wo, ms)
    res = run_bass_kernel_spmd(nc, in_maps, list(range(8)))

    out = np.empty((B, L, D), np.float32)
    for b in range(B):
        acc = res.results[4 * b]["yt"].astype(np.float32)
        for g in range(1, 4):
            acc = acc + res.results[4 * b + g]["yt"]
        out[b] = acc.T + bo
    return out
